# revision 23
# baseline (speedup 1.0000x reference)
"""Trainium2 Bass kernel for a Conv-TasNet-style decoder (mask * wave ->
overlap_and_add -> trim).

Reference computation (per batch element b):
    A[c, d, t] = x[b, c, d, t] * x_wave[b, d, t]          (broadcast over c)
    frames     = A transposed to [c, t, d]  (frame length D=16, hop 8)
    unsliced   = overlap_and_add(frames, 8)               # [c, (T+1)*8]
    y          = unsliced[:, pad_left : -pad_right]

With hop=8 and D=16 the overlap-add decomposes into two interleaved
streams; for the middle region (everything when pad_left = pad_right = 8):

    y[c][8s + r] = x[c, r, s+1]*w[r, s+1] + x[c, r+8, s]*w[r+8, s]

i.e. purely elementwise over s plus an 8-way interleave.

Device strategy (per core = per batch element):
  * fp16 end-to-end on the device: halves HBM traffic (the dominant
    cost; this problem is memory-bound) at ~4e-4 relative error.
  * The host pre-packs x and x_wave into the exact SBUF layout the
    kernel consumes: partition p owns frames [1000p, 1000(p+1)), and
    the free dim is already interleaved as q = 8j + r (with the
    low-stream +1 frame shift folded in).  Every DMA descriptor is
    then a >=4 KB contiguous run and every DVE access pattern is
    unit-stride, which enables the 2x fp16 dual-pipe vector mode.
  * Per speaker: two tensor_mul + one tensor_add on [128, 8000] fp16
    (chunked in q: 2000/4000/2000 for a fast ramp and short tail).
  * Only the two HWDGE rings (sync + scalar) move data - loads first,
    stores queued behind them (two rings already saturate the HBM
    share, and SWDGE descriptor-ring traffic slows SDMA engines 7/15,
    which lengthens every DMA's completion semaphore).  Every tile
    gets a private buffer so no WAR semaphores exist, and the
    measured span is bottlenecked only by the slowest SDMA engine.

Sharding: pure data parallel - core b computes batch element b (B=8
matches the 8 NeuronCores); no cross-core communication.
"""

import numpy as np

_B, _C, _D, _T = 8, 2, 16, 128000
_HOP = 8
_S = _T * _HOP            # padded per-speaker device output length (1024000)
_MID = _S - _HOP          # valid middle length (1023992)
_P = 128                  # SBUF partitions
_Q = _S // _P             # interleaved elements per partition (8000)
# q-chunks: small first chunk (fast compute ramp) and small last chunk
# (short store tail); the middle chunk keeps 8 KB descriptors.
_CH = [(0, 2000), (2000, 4000), (6000, 2000)]

_cached = None            # (nc, run_bass_kernel_spmd)


def _build():
    """Build the Bass module (one NeuronCore's program). Cached."""
    global _cached
    if _cached is not None:
        return _cached

    import concourse.bacc as bacc
    import concourse.mybir as mybir
    import concourse.tile as tile
    from concourse.bass_utils import run_bass_kernel_spmd

    f16 = mybir.dt.float16
    P, Q, CH = _P, _Q, _CH

    nc = bacc.Bacc(debug=False)
    # Host-prepacked inputs: [c, side, p, q] and [side, p, q] with
    # q = 8j + r already interleaved (side 0 = low stream, +1 frame
    # shift baked in; side 1 = high stream).
    xin = nc.declare_dram_parameter("xin", [_C, 2, P, Q], f16, isOutput=False)
    win = nc.declare_dram_parameter("win", [2, P, Q], f16, isOutput=False)
    y = nc.declare_dram_parameter("y", [_C, P, Q], f16, isOutput=True)

    xf = xin[:].rearrange("c s p q -> (c s p q)")
    wf = win[:].rearrange("s p q -> (s p q)")
    yf = y[:].rearrange("c p q -> (c p q)")

    def pq_view(flat, idx):
        # [p, q] view of block `idx` (blocks of P*Q elements)
        return flat[idx * P * Q : (idx + 1) * P * Q].rearrange(
            "(p q) -> p q", p=P
        )

    with tile.TileContext(nc) as tc:
        with (
            tc.tile_pool(name="wpool", bufs=1) as wpool,
            tc.tile_pool(name="xpool", bufs=6) as xpool,
            tc.tile_pool(name="ppool", bufs=1) as ppool,
            tc.tile_pool(name="zpool", bufs=6) as zpool,
        ):
            # Phase 1: issue every load up front (program order == ring
            # order == sem-lane order, so no compute-dependent wait can
            # block a ring or the DVE stream behind slow transfers).
            #   sync   : wl chunks + xl chunks (W ahead of the x chunk
            #            it gates), then half the stores at the tail
            #   scalar : wh + xh, then the other half of the stores
            # Every tile gets a private buffer (no reuse -> no WAR
            # semaphores; DVE in-order execution makes ppool reuse free).
            wl_t = [None] * len(CH)
            wh_t = [None] * len(CH)
            xl_t = {}
            xh_t = {}
            NQ = len(CH)
            for c in range(_C):
                xl_v = pq_view(xf, 2 * c)      # low stream of speaker c
                xh_v = pq_view(xf, 2 * c + 1)  # high stream
                for qi, (q0, qc) in enumerate(CH):
                    sl = slice(q0, q0 + qc)
                    if c == 0:
                        wl_t[qi] = wpool.tile([P, qc], f16, tag=f"wl{qi}",
                                              name=f"wl{qi}")
                        wh_t[qi] = wpool.tile([P, qc], f16, tag=f"wh{qi}",
                                              name=f"wh{qi}")
                        nc.sync.dma_start(
                            out=wl_t[qi][:], in_=pq_view(wf, 0)[:, sl]
                        )
                        nc.scalar.dma_start(
                            out=wh_t[qi][:], in_=pq_view(wf, 1)[:, sl]
                        )
                    xl_t[c, qi] = xpool.tile([P, 4000], f16, tag="xl",
                                             name="xlt")[:, :qc]
                    nc.sync.dma_start(out=xl_t[c, qi][:], in_=xl_v[:, sl])
                    xh_t[c, qi] = xpool.tile([P, 4000], f16, tag="xh",
                                             name="xht")[:, :qc]
                    nc.scalar.dma_start(out=xh_t[c, qi][:], in_=xh_v[:, sl])

            # Phase 2: compute + stores, chunk by chunk.  Stores ride
            # the same two HWDGE rings, queued behind the loads
            # (2 rings already saturate HBM, so serializing stores
            # after loads costs nothing) - SWDGE stays completely idle
            # because its descriptor-ring traffic slows SDMA engines
            # 7/15 and with it every load's completion semaphore.
            for c in range(_C):
                y_v = pq_view(yf, c)
                for qi, (q0, qc) in enumerate(CH):
                    sl = slice(q0, q0 + qc)
                    # All-unit-stride fp16 ops -> DVE 2x dual-pipe mode.
                    pl = ppool.tile([P, 4000], f16, tag="pl", name="pl")[:, :qc]
                    nc.vector.tensor_mul(pl[:], xl_t[c, qi][:], wl_t[qi][:])
                    ph = ppool.tile([P, 4000], f16, tag="ph", name="ph")[:, :qc]
                    nc.vector.tensor_mul(ph[:], xh_t[c, qi][:], wh_t[qi][:])
                    zt = zpool.tile([P, 4000], f16, tag="zt", name="zt")[:, :qc]
                    nc.vector.tensor_add(zt[:], pl[:], ph[:])

                    if c == _C - 1 and qi >= NQ - 2:
                        # Tail stores: split across both rings so the
                        # final drains run in parallel.
                        h = qc // 2
                        nc.sync.dma_start(
                            out=y_v[:, q0 : q0 + h], in_=zt[:, :h]
                        )
                        nc.scalar.dma_start(
                            out=y_v[:, q0 + h : q0 + qc], in_=zt[:, h:]
                        )
                    else:
                        seng = nc.sync if (c * NQ + qi) % 2 == 0 else nc.scalar
                        seng.dma_start(out=y_v[:, sl], in_=zt[:])

    nc.compile()

    _cached = (nc, run_bass_kernel_spmd)
    return _cached


def _prepack(x, w):
    """Pack [B,C,16,T] x and [B,16,T] w into the device layout.

    Returns xin [B, C, 2, P, Q] fp16 and win [B, 2, P, Q] fp16 where
    [p, 8j+r] = stream[r, 1000p + j]; low stream is shifted one frame
    (frame s+1) and zero-padded at the end (that output lands in the
    trimmed tail).
    """
    B, C, D, T = _B, _C, _D, _T
    JB = _T // _P  # frames per partition (1000)

    def pack(rows):  # [..., 8, T] -> [..., P, Q] with q = 8j + r
        sh = rows.shape[:-2]
        out = rows.reshape(*sh, 8, _P, JB)
        out = np.moveaxis(out, -3, -1)          # [..., P, JB, 8]
        return np.ascontiguousarray(out).reshape(*sh, _P, _Q)

    xl = np.zeros((B, C, 8, T), np.float16)
    xl[..., : T - 1] = x[:, :, 0:8, 1:]
    xh = x[:, :, 8:16, :].astype(np.float16)
    wl = np.zeros((B, 8, T), np.float16)
    wl[..., : T - 1] = w[:, 0:8, 1:]
    wh = w[:, 8:16, :].astype(np.float16)

    xin = np.stack([pack(xl), pack(xh)], axis=2)   # [B, C, 2, P, Q]
    win = np.stack([pack(wl), pack(wh)], axis=1)   # [B, 2, P, Q]
    return xin, win


def _run_device(x, w, trace=False):
    nc, run_bass_kernel_spmd = _build()
    xin, win = _prepack(x, w)
    in_maps = [
        {"xin": np.ascontiguousarray(xin[b]),
         "win": np.ascontiguousarray(win[b])}
        for b in range(_B)
    ]
    res = run_bass_kernel_spmd(nc, in_maps, core_ids=list(range(_B)), trace=trace)
    mid = np.stack(
        [r["y"].reshape(_C, _S)[:, :_MID].astype(np.float32) for r in res.results]
    )
    return mid, res


def kernel(x, x_wave, pad_left=8, pad_right=8, _trace=False, _return_res=False):
    x = np.asarray(x, dtype=np.float32)
    w = np.asarray(x_wave, dtype=np.float32)
    pl, pr = int(pad_left), int(pad_right)
    assert x.shape == (_B, _C, _D, _T) and w.shape == (_B, _D, _T)

    mid, res = _run_device(x, w, trace=_trace)

    if pl == 8 and pr == 8:
        out = mid
    else:
        # General trim: reconstruct the 8 leading / 8 trailing elements
        # of the unsliced overlap-add on the host (they only involve the
        # first/last frame) and slice.
        front = x[:, :, 0:8, 0] * w[:, None, 0:8, 0]        # unsliced[0:8]
        back = x[:, :, 8:16, -1] * w[:, None, 8:16, -1]     # unsliced[-8:]
        full = np.concatenate([front, mid, back], axis=-1)  # [B, C, (T+1)*8]
        end = full.shape[-1] - pr
        out = np.ascontiguousarray(full[:, :, pl:end])

    if _return_res:
        return out, res
    return out
